# revision 51
# baseline (speedup 1.0000x reference)
"""Bass/Tile GRU kernel for trn2, data-parallel over batch on 8 cores.

Model: xe = emb[x]; gi = xe @ w_ih.T + b_ih (+ b_hh for r,z); per step:
  gh = h @ w_hh.T
  r = sig(gi_r + gh_r); z = sig(gi_z + gh_z)
  n = tanh(gi_n + r * (gh_n + bhh_n)); h = (1-z)*n + z*h
then logits = outs @ fc_w.T + fc_b; out = log_softmax(logits)

Cost-model-driven layout (matmul cost = out free size; stationary side free):
all recurrence tensors live transposed, partition = hidden/gate-dim-in-tile,
free = (tile index k or m, batch b).  Per-core B_loc = 8.

  h      [128, 64]   h[j, b] at partition j%128, free (j//128)*8 + b
  psum   [128, 192]  ghT[g*1024+j, b] at partition j%128, free m*8+b,
                     m = (g*1024+j)//128  (m 0..7 r, 8..15 z, 16..23 n)
  gi_sb  [128, 192]  same layout as psum (from phase 1, via DRAM)

Recurrence matmuls are weights-stationary: lhsT = w_hh.T tile [128, 128]
(m,k), rhs = h slice [128, 8] -> 192 matmuls of out-free 8 per step, plus
two identity-lhsT fold matmuls that accumulate gi (r,z) and bhh_n into
psum.  h_new is produced directly in the h layout - no transposes.
"""

import numpy as np
from contextlib import ExitStack

import concourse.bass as bass
import concourse.tile as tile
from concourse import bacc, mybir

F32 = mybir.dt.float32
BF16 = mybir.dt.bfloat16
I16 = mybir.dt.int16
AF = mybir.ActivationFunctionType

VOCAB, D_IN, D_H, D_OUT, B, S = 32000, 512, 1024, 64, 64, 256
P = 128
NCORES = 8
BL = B // NCORES          # 8 batch rows per core
NT = BL * S               # 2048 (b,t) rows per core
FLUSH = 32                # steps between h flushes to DRAM
NM = 3 * D_H // P         # 24 gate-row tiles
KH = D_H // P             # 8 hidden k-tiles
KI = D_IN // P            # 4 input k-tiles
RC = 256                  # phase-1 row-chunk (32 steps * 8 batch)
NCH = NT // RC            # 8 phase-1 chunks


def host_prep(x, emb, w_ih, w_hh, b_ih, b_hh, fc_w, fc_b):
    """Produce the per-core and shared input arrays for the bass kernel."""
    import ml_dtypes
    emb_bf = emb.astype(ml_dtypes.bfloat16)
    # lhsT layouts: [p, k, m, mj]  ->  w[128*m+mj, 128*k+p]
    wih_l = np.ascontiguousarray(
        w_ih.T.reshape(KI, P, NM, P).transpose(1, 0, 2, 3)
    ).astype(ml_dtypes.bfloat16)
    whh_l = np.ascontiguousarray(
        w_hh.T.reshape(KH, P, NM, P).transpose(1, 0, 2, 3)
    ).astype(ml_dtypes.bfloat16)
    # bias for gi: b_ih everywhere, plus b_hh for the r,z gates (m < 16)
    bgiT = b_ih.reshape(NM, P).T.copy()
    bgiT[:, :16] += b_hh.reshape(NM, P).T[:, :16]
    bgiT = bgiT.astype(np.float32)  # [128, 24]
    # bhh_n expanded over batch: [p, 8k+b] = b_hh[2H + 128k + p]
    bhnE = np.repeat(
        b_hh[2 * D_H:].reshape(KH, P).T[:, :, None], BL, axis=2
    ).reshape(P, KH * BL).astype(ml_dtypes.bfloat16)  # [128, 64]
    fcw_l = np.ascontiguousarray(
        fc_w.T.reshape(KH, P, D_OUT)).astype(ml_dtypes.bfloat16)
    fcb = fc_b.astype(np.float32).reshape(D_OUT, 1)
    iden_bf = np.eye(P, dtype=np.float32).astype(ml_dtypes.bfloat16)
    iden32 = np.eye(P, dtype=np.float32)

    shared = dict(
        emb=emb_bf, wih=wih_l, whh=whh_l, bgi=bgiT, bhn=bhnE,
        fcw=fcw_l, fcb=fcb, iden=iden_bf, iden32=iden32,
    )
    per_core = []
    for c in range(NCORES):
        ids = np.ascontiguousarray(
            np.asarray(x[c * BL:(c + 1) * BL, :S]).T).reshape(-1).astype(np.int16)
        tmp = np.zeros((16, P), np.int16)
        i = np.arange(NT)
        li = i % 512
        tmp[li % 16, (i // 512) * 32 + li // 16] = ids
        idx = np.tile(tmp, (8, 1))  # replicated for the 8 Q7 cores
        per_core.append({"idx": idx, **shared})
    return per_core


def build_kernel():
    nc = bacc.Bacc("TRN2", debug=False, num_devices=1)

    idx = nc.dram_tensor("idx", [P, P], I16, kind="ExternalInput").ap()
    emb = nc.dram_tensor("emb", [VOCAB, D_IN], BF16, kind="ExternalInput").ap()
    wih = nc.dram_tensor("wih", [P, KI, NM, P], BF16, kind="ExternalInput").ap()
    whh = nc.dram_tensor("whh", [P, KH, NM, P], BF16, kind="ExternalInput").ap()
    bgi = nc.dram_tensor("bgi", [P, NM], F32, kind="ExternalInput").ap()
    bhn = nc.dram_tensor("bhn", [P, KH * BL], BF16, kind="ExternalInput").ap()
    fcw = nc.dram_tensor("fcw", [KH, P, D_OUT], BF16, kind="ExternalInput").ap()
    fcb = nc.dram_tensor("fcb", [D_OUT, 1], F32, kind="ExternalInput").ap()
    iden = nc.dram_tensor("iden", [P, P], BF16, kind="ExternalInput").ap()
    iden32 = nc.dram_tensor("iden32", [P, P], F32, kind="ExternalInput").ap()
    out = nc.dram_tensor("out", [NT, D_OUT], F32, kind="ExternalOutput").ap()

    with tile.TileContext(nc) as tc, ExitStack() as ctx:
        singles = ctx.enter_context(tc.tile_pool(name="singles", bufs=1))
        dram = ctx.enter_context(tc.tile_pool(name="dram", bufs=1, space="DRAM"))

        # ---- persistent SBUF state ----
        # order: what phase-1 chunk 0 needs comes first (idx, wih, bgi,
        # gathers); whh (6 MB, ~19 us) only gates step 0.
        idx_sb = singles.tile([P, P], I16)
        nc.sync.dma_start(idx_sb[:], idx)
        wih_sb = singles.tile([P, KI, NM, P], BF16)
        for mg in range(3):  # split so the first p1 units start sooner
            nc.sync.dma_start(wih_sb[:, :, 8 * mg:8 * mg + 8, :],
                              wih[:, :, 8 * mg:8 * mg + 8, :])
        bgi_sb = singles.tile([P, NM], F32)
        nc.sync.dma_start(bgi_sb[:], bgi)
        iden_sb = singles.tile([P, P], BF16)
        nc.sync.dma_start(iden_sb[:], iden)

        # gathered embeddings, transposed: xeT[p, gc, kc, i] = xe[512gc+i, 128kc+p]
        xeT = singles.tile([P, NT // 512, KI, 512], BF16)
        for gc in range(NT // 512):
            nc.gpsimd.dma_gather(
                out_ap=xeT[:, gc],
                in_ap=emb,
                idxs_ap=idx_sb[:, gc * 32:(gc + 1) * 32],
                num_idxs=512,
                num_idxs_reg=512,
                elem_size=D_IN,
                transpose=True,
            )

        # big weights go via the ACT hwdge queue, parallel to SP's wih
        whh_sb = singles.tile([P, KH, NM, P], BF16)
        nc.scalar.dma_start(whh_sb[:], whh)
        bhn_sb = singles.tile([P, KH * BL], BF16)
        nc.scalar.dma_start(bhn_sb[:], bhn)
        fcw_sb = singles.tile([P, KH, D_OUT], BF16)
        nc.scalar.dma_start(fcw_sb[:], fcw.rearrange("k p c -> p k c"))
        fcb_sb = singles.tile([D_OUT, 1], F32)
        nc.scalar.dma_start(fcb_sb[:], fcb)
        iden32_sb = singles.tile([P, P], F32)
        nc.scalar.dma_start(iden32_sb[:], iden32)

        # all hidden states live in SBUF: outs_sb[p, k, t, b] = h_t[128k+p, b]
        outs_sb = singles.tile([P, KH, S, BL], BF16)

        # ---- phase 1: giT = (xe @ w_ih.T + bias), written straight into a
        # double-buffered SBUF chunk (64 steps per chunk, no DRAM round-trip).
        TCH = 512 // BL  # 64 steps per chunk
        gi_buf = singles.tile([P, 2, TCH, NM, BL], BF16)
        p2ctx = ExitStack()
        p1psum = p2ctx.enter_context(tc.tile_pool(name="p1psum", bufs=2, space="PSUM"))

        def emit_p1_mms(c, m, r0, nr):
            # rows 512c+r0.. (+nr), gate-row tile m
            tag = f"p1ps{nr}"
            ps = p1psum.tile([P, nr], F32, tag=tag, name=f"p1ps_{c}_{m}_{r0}")
            for k in range(KI):
                nc.tensor.matmul(
                    ps[:],
                    lhsT=wih_sb[:, k, m, :],
                    rhs=xeT[:, c, k, r0:r0 + nr],
                    start=(k == 0), stop=(k == KI - 1),
                )
            return ps

        def emit_p1_bias(c, m, r0, nr, ps, on_act=False):
            dst = gi_buf[:, c % 2, r0 // BL:(r0 + nr) // BL, m, :]
            src = ps.rearrange("p (t b) -> p t b", b=BL)
            if on_act:
                nc.scalar.activation(dst, src, AF.Identity,
                                     bias=bgi_sb[:, m:m + 1])
            else:
                nc.vector.tensor_scalar_add(dst, src, bgi_sb[:, m:m + 1])

        # Chunk 0 is built in 4 waves of 16 steps: wave 0 before the loop,
        # waves 1-3 during early steps.  Chunks 1-3 stream in as half-units
        # during the preceding chunk.  mms go one step ahead of the bias so
        # the bias's psum wait is satisfied when the DVE sequencer hits it.
        p1_mm_sched, p1_bias_sched = {}, {}
        for w in range(1, 4):
            for m in range(NM):
                t0 = 8 * (w - 1) + m // 3
                p1_mm_sched.setdefault(t0, []).append((0, m, 128 * w, 128))
                p1_bias_sched.setdefault(t0 + 1, []).append((0, m, 128 * w, 128))
        for c in range(1, S // TCH):
            for u in range(2 * NM):
                t0 = TCH * (c - 1) + (4 * u) // 3
                p1_mm_sched.setdefault(t0, []).append((c, u // 2, 256 * (u % 2), 256))
                p1_bias_sched.setdefault(t0 + 1, []).append((c, u // 2, 256 * (u % 2), 256))
        p1_pending = {}
        ps_q = []
        for m in range(NM):
            ps_q.append((m, emit_p1_mms(0, m, 0, 128)))
            if len(ps_q) == 3:
                pm, pps = ps_q.pop(0)
                emit_p1_bias(0, pm, 0, 128, pps, on_act=(pm % 2 == 1))
        for pm, pps in ps_q:
            emit_p1_bias(0, pm, 0, 128, pps, on_act=(pm % 2 == 1))

        # ---- phase 2: recurrence ----
        gpool = ctx.enter_context(tc.tile_pool(name="gates", bufs=3))
        psg = p2ctx.enter_context(tc.tile_pool(name="psg", bufs=1, space="PSUM"))

        h0 = singles.tile([P, KH, BL], BF16)
        nc.vector.memset(h0[:], 0.0)
        h_prev = h0[:]  # [P, KH, BL]

        for t in range(S):
            gi_sb = gi_buf[:, (t // TCH) % 2, t % TCH].rearrange(
                "p m b -> p (m b)")  # [P, 192]

            # separate psum tiles per gate; burst ordered so ps_r lands
            # first (starts the ACT chain), then ps_n (t1), then ps_z.
            ps_r = psg.tile([P, 64], F32, tag="ps_r")
            ps_z = psg.tile([P, 64], F32, tag="ps_z")
            ps_n = psg.tile([P, 64], F32, tag="ps_n")

            # psum bank zeroing is lazy at whole-bank granularity on a
            # start=True, so the fold must come FIRST (start=True over the
            # whole tile); all gh matmuls then accumulate with start=False.
            def mm_group(pst, m0, fold_rhs):
                nc.tensor.matmul(pst[:], lhsT=iden_sb[:], rhs=fold_rhs,
                                 start=True, stop=False, skip_group_check=True)
                for m in range(m0, m0 + 8):
                    rel = BL * (m - m0)
                    for k in range(KH):
                        nc.tensor.matmul(
                            pst[:, rel:rel + BL],
                            lhsT=whh_sb[:, k, m, :],
                            rhs=h_prev[:, k, :],
                            start=False, stop=(m == m0 + 7 and k == KH - 1),
                            skip_group_check=True,
                        )

            mm_group(ps_r, 0, gi_sb[:, 0:64])
            mm_group(ps_n, 16, bhn_sb[:])
            mm_group(ps_z, 8, gi_sb[:, 64:128])

            # gates: ACT order r, z, tanh; critical chain
            # r -> t1 -> t2 -> tanh -> v -> h'  (t1 pure-SBUF via nb copy)
            r_g = gpool.tile([P, 64], BF16, tag="rg")
            nc.scalar.activation(r_g[:], ps_r[:], AF.Sigmoid)
            z_g = gpool.tile([P, 64], BF16, tag="zg")
            nc.scalar.activation(z_g[:], ps_z[:], AF.Sigmoid)
            t1 = gpool.tile([P, 64], BF16, tag="t1")
            nc.vector.tensor_mul(t1[:], r_g[:], ps_n[:])
            t2 = gpool.tile([P, 64], BF16, tag="t2")
            nc.vector.tensor_add(t2[:], t1[:], gi_sb[:, 128:192])
            n_g = gpool.tile([P, 64], BF16, tag="n")
            nc.scalar.activation(n_g[:], t2[:], AF.Tanh)
            omz = gpool.tile([P, 64], BF16, tag="omz")
            nc.vector.tensor_scalar(omz[:], z_g[:], -1.0, 1.0,
                                    op0=mybir.AluOpType.mult,
                                    op1=mybir.AluOpType.add)
            zh = gpool.tile([P, KH, BL], BF16, tag="zh")
            nc.vector.tensor_mul(
                zh[:], z_g[:].rearrange("p (k b) -> p k b", b=BL), h_prev)
            v_g = gpool.tile([P, 64], BF16, tag="v")
            nc.vector.tensor_mul(v_g[:], omz[:], n_g[:])
            h_new = outs_sb[:, :, t, :]
            nc.vector.tensor_add(
                h_new, v_g[:].rearrange("p (k b) -> p k b", b=BL), zh[:])

            # phase-1 work last: it then sits after this step's critical
            # ops in engine order
            for key in p1_bias_sched.get(t, ()):
                emit_p1_bias(*key, p1_pending.pop(key))
            for key in p1_mm_sched.get(t, ()):
                p1_pending[key] = emit_p1_mms(*key)

            h_prev = h_new

        p2ctx.close()

        # ---- phase 3: fc + log_softmax, batched per 512-row chunk ----
        # |logits| < ~4 so exp() is safe without the max-subtraction;
        # log_softmax = x - ln(sum(exp(x))).  Two passes keep Exp and Ln
        # table loads from thrashing.
        NQ = NT // 512
        with tc.tile_pool(name="p3", bufs=2) as p3, \
             tc.tile_pool(name="p3b", bufs=1) as p3b, \
             tc.tile_pool(name="p3psum", bufs=2, space="PSUM") as p3psum, \
             tc.tile_pool(name="p3psum2", bufs=1, space="PSUM") as p3psum2:
            lgps = [p3psum2.tile([P, 4, D_OUT], F32, tag=f"lgps{q}",
                                 name=f"lgps{q}") for q in range(NQ)]
            sms = [p3b.tile([P, 4], F32, tag=f"sm{q}", name=f"sm{q}")
                   for q in range(NQ)]
            for q in range(NQ):
                ps3 = p3psum.tile([D_OUT, 512], F32)
                for k in range(KH):
                    nc.tensor.matmul(
                        ps3[:],
                        lhsT=fcw_sb[:, k, :],
                        rhs=outs_sb[:, k, TCH * q:TCH * q + TCH, :].rearrange(
                            "p t b -> p (t b)"),
                        start=(k == 0), stop=(k == KH - 1),
                    )
                logitsT = p3.tile([D_OUT, 512], F32, tag="logitsT")
                nc.vector.tensor_scalar_add(logitsT[:], ps3[:], fcb_sb[:])
                for w in range(4):
                    nc.tensor.transpose(
                        lgps[q][:, w, :], logitsT[:, w * 128:w * 128 + 128],
                        iden32_sb[0:D_OUT, 0:D_OUT],
                    )
                ex = p3.tile([P, 4, D_OUT], BF16, tag="ex")
                nc.scalar.activation(ex[:], lgps[q][:], AF.Exp)
                nc.vector.reduce_sum(sms[q][:], ex[:], axis=mybir.AxisListType.X)
            for q in range(NQ):
                lsm = p3.tile([P, 4], F32, tag="lsm")
                nc.scalar.activation(lsm[:], sms[q][:], AF.Ln)
                lb = lsm[:]
                res = p3.tile([P, 4, D_OUT], F32, tag="res")
                nc.vector.tensor_sub(
                    res[:], lgps[q][:],
                    bass.AP(tensor=lb.tensor, offset=lb.offset,
                            ap=[list(lb.ap[0]), list(lb.ap[1]), [0, D_OUT]]),
                )
                nc.sync.dma_start(
                    bass.AP(tensor=out.tensor, offset=out.offset + q * 512 * D_OUT,
                            ap=[[D_OUT, P], [P * D_OUT, 4], [1, D_OUT]]),
                    res[:],
                )

    nc.compile()
    return nc


def assemble_output(core_outs):
    full = np.zeros((B, S, D_OUT), np.float32)
    for c, o in enumerate(core_outs):
        o = o.reshape(S, BL, D_OUT)  # rows are t-major: n = t*8 + b
        full[c * BL:(c + 1) * BL] = o.transpose(1, 0, 2)
    return full


# ----------------------------------------------------------------------------
# Harness entry point: kernel(**inputs) -> [B, S, D_OUT] float32
# ----------------------------------------------------------------------------
_CACHE = {}


def _get_nc():
    if "nc" not in _CACHE:
        _CACHE["nc"] = build_kernel()
    return _CACHE["nc"]


def kernel(x, emb, w_ih, w_hh, b_ih, b_hh, fc_w, fc_b):
    x = np.asarray(x)
    emb = np.asarray(emb, np.float32)
    w_ih = np.asarray(w_ih, np.float32)
    w_hh = np.asarray(w_hh, np.float32)
    b_ih = np.asarray(b_ih, np.float32)
    b_hh = np.asarray(b_hh, np.float32)
    fc_w = np.asarray(fc_w, np.float32)
    fc_b = np.asarray(fc_b, np.float32)

    from concourse.bass_utils import run_bass_kernel_spmd

    per_core = host_prep(x, emb, w_ih, w_hh, b_ih, b_hh, fc_w, fc_b)
    nc = _get_nc()
    res = run_bass_kernel_spmd(
        nc, per_core, core_ids=list(range(NCORES)), trace=False
    )
    return assemble_output([r["out"] for r in res.results])


# revision 52
# speedup vs baseline: 1.0002x; 1.0002x over previous
"""Bass/Tile GRU kernel for trn2, data-parallel over batch on 8 cores.

Model: xe = emb[x]; gi = xe @ w_ih.T + b_ih (+ b_hh for r,z); per step:
  gh = h @ w_hh.T
  r = sig(gi_r + gh_r); z = sig(gi_z + gh_z)
  n = tanh(gi_n + r * (gh_n + bhh_n)); h = (1-z)*n + z*h
then logits = outs @ fc_w.T + fc_b; out = log_softmax(logits)

Cost-model-driven layout (matmul cost = out free size; stationary side free):
all recurrence tensors live transposed, partition = hidden/gate-dim-in-tile,
free = (tile index k or m, batch b).  Per-core B_loc = 8.

  h      [128, 64]   h[j, b] at partition j%128, free (j//128)*8 + b
  psum   [128, 192]  ghT[g*1024+j, b] at partition j%128, free m*8+b,
                     m = (g*1024+j)//128  (m 0..7 r, 8..15 z, 16..23 n)
  gi_sb  [128, 192]  same layout as psum (from phase 1, via DRAM)

Recurrence matmuls are weights-stationary: lhsT = w_hh.T tile [128, 128]
(m,k), rhs = h slice [128, 8] -> 192 matmuls of out-free 8 per step, plus
two identity-lhsT fold matmuls that accumulate gi (r,z) and bhh_n into
psum.  h_new is produced directly in the h layout - no transposes.
"""

import numpy as np
from contextlib import ExitStack

import concourse.bass as bass
import concourse.tile as tile
from concourse import bacc, mybir

F32 = mybir.dt.float32
BF16 = mybir.dt.bfloat16
I16 = mybir.dt.int16
AF = mybir.ActivationFunctionType

VOCAB, D_IN, D_H, D_OUT, B, S = 32000, 512, 1024, 64, 64, 256
P = 128
NCORES = 8
BL = B // NCORES          # 8 batch rows per core
NT = BL * S               # 2048 (b,t) rows per core
FLUSH = 32                # steps between h flushes to DRAM
NM = 3 * D_H // P         # 24 gate-row tiles
KH = D_H // P             # 8 hidden k-tiles
KI = D_IN // P            # 4 input k-tiles
RC = 256                  # phase-1 row-chunk (32 steps * 8 batch)
NCH = NT // RC            # 8 phase-1 chunks


def host_prep(x, emb, w_ih, w_hh, b_ih, b_hh, fc_w, fc_b):
    """Produce the per-core and shared input arrays for the bass kernel."""
    import ml_dtypes
    emb_bf = emb.astype(ml_dtypes.bfloat16)
    # lhsT layouts: [p, k, m, mj]  ->  w[128*m+mj, 128*k+p]
    wih_l = np.ascontiguousarray(
        w_ih.T.reshape(KI, P, NM, P).transpose(1, 0, 2, 3)
    ).astype(ml_dtypes.bfloat16)
    whh_l = np.ascontiguousarray(
        w_hh.T.reshape(KH, P, NM, P).transpose(1, 0, 2, 3)
    ).astype(ml_dtypes.bfloat16)
    # bias for gi: b_ih everywhere, plus b_hh for the r,z gates (m < 16)
    bgiT = b_ih.reshape(NM, P).T.copy()
    bgiT[:, :16] += b_hh.reshape(NM, P).T[:, :16]
    bgiT = bgiT.astype(np.float32)  # [128, 24]
    # bhh_n expanded over batch: [p, 8k+b] = b_hh[2H + 128k + p]
    bhnE = np.repeat(
        b_hh[2 * D_H:].reshape(KH, P).T[:, :, None], BL, axis=2
    ).reshape(P, KH * BL).astype(ml_dtypes.bfloat16)  # [128, 64]
    fcw_l = np.ascontiguousarray(
        fc_w.T.reshape(KH, P, D_OUT)).astype(ml_dtypes.bfloat16)
    fcb = fc_b.astype(np.float32).reshape(D_OUT, 1)
    iden_bf = np.eye(P, dtype=np.float32).astype(ml_dtypes.bfloat16)
    iden32 = np.eye(P, dtype=np.float32)

    shared = dict(
        emb=emb_bf, wih=wih_l, whh=whh_l, bgi=bgiT, bhn=bhnE,
        fcw=fcw_l, fcb=fcb, iden=iden_bf, iden32=iden32,
    )
    per_core = []
    for c in range(NCORES):
        ids = np.ascontiguousarray(
            np.asarray(x[c * BL:(c + 1) * BL, :S]).T).reshape(-1).astype(np.int16)
        tmp = np.zeros((16, P), np.int16)
        i = np.arange(NT)
        li = i % 512
        tmp[li % 16, (i // 512) * 32 + li // 16] = ids
        idx = np.tile(tmp, (8, 1))  # replicated for the 8 Q7 cores
        per_core.append({"idx": idx, **shared})
    return per_core


def build_kernel():
    nc = bacc.Bacc("TRN2", debug=False, num_devices=1)

    idx = nc.dram_tensor("idx", [P, P], I16, kind="ExternalInput").ap()
    emb = nc.dram_tensor("emb", [VOCAB, D_IN], BF16, kind="ExternalInput").ap()
    wih = nc.dram_tensor("wih", [P, KI, NM, P], BF16, kind="ExternalInput").ap()
    whh = nc.dram_tensor("whh", [P, KH, NM, P], BF16, kind="ExternalInput").ap()
    bgi = nc.dram_tensor("bgi", [P, NM], F32, kind="ExternalInput").ap()
    bhn = nc.dram_tensor("bhn", [P, KH * BL], BF16, kind="ExternalInput").ap()
    fcw = nc.dram_tensor("fcw", [KH, P, D_OUT], BF16, kind="ExternalInput").ap()
    fcb = nc.dram_tensor("fcb", [D_OUT, 1], F32, kind="ExternalInput").ap()
    iden = nc.dram_tensor("iden", [P, P], BF16, kind="ExternalInput").ap()
    iden32 = nc.dram_tensor("iden32", [P, P], F32, kind="ExternalInput").ap()
    out = nc.dram_tensor("out", [NT, D_OUT], F32, kind="ExternalOutput").ap()

    with tile.TileContext(nc) as tc, ExitStack() as ctx:
        singles = ctx.enter_context(tc.tile_pool(name="singles", bufs=1))
        dram = ctx.enter_context(tc.tile_pool(name="dram", bufs=1, space="DRAM"))

        # ---- persistent SBUF state ----
        # order: what phase-1 chunk 0 needs comes first (idx, wih, bgi,
        # gathers); whh (6 MB, ~19 us) only gates step 0.
        idx_sb = singles.tile([P, P], I16)
        nc.sync.dma_start(idx_sb[:], idx)
        wih_sb = singles.tile([P, KI, NM, P], BF16)
        for mg in range(3):  # split so the first p1 units start sooner
            nc.sync.dma_start(wih_sb[:, :, 8 * mg:8 * mg + 8, :],
                              wih[:, :, 8 * mg:8 * mg + 8, :])
        bgi_sb = singles.tile([P, NM], F32)
        nc.sync.dma_start(bgi_sb[:], bgi)
        iden_sb = singles.tile([P, P], BF16)
        nc.sync.dma_start(iden_sb[:], iden)

        # gathered embeddings, transposed: xeT[p, gc, kc, i] = xe[512gc+i, 128kc+p]
        xeT = singles.tile([P, NT // 512, KI, 512], BF16)
        for gc in range(NT // 512):
            nc.gpsimd.dma_gather(
                out_ap=xeT[:, gc],
                in_ap=emb,
                idxs_ap=idx_sb[:, gc * 32:(gc + 1) * 32],
                num_idxs=512,
                num_idxs_reg=512,
                elem_size=D_IN,
                transpose=True,
            )

        # big weights go via the ACT hwdge queue, parallel to SP's wih
        whh_sb = singles.tile([P, KH, NM, P], BF16)
        nc.scalar.dma_start(whh_sb[:], whh)
        bhn_sb = singles.tile([P, KH * BL], BF16)
        nc.scalar.dma_start(bhn_sb[:], bhn)
        fcw_sb = singles.tile([P, KH, D_OUT], BF16)
        nc.scalar.dma_start(fcw_sb[:], fcw.rearrange("k p c -> p k c"))
        fcb_sb = singles.tile([D_OUT, 1], F32)
        nc.scalar.dma_start(fcb_sb[:], fcb)
        iden32_sb = singles.tile([P, P], F32)
        nc.scalar.dma_start(iden32_sb[:], iden32)

        # all hidden states live in SBUF: outs_sb[p, k, t, b] = h_t[128k+p, b]
        outs_sb = singles.tile([P, KH, S, BL], BF16)

        # ---- phase 1: giT = (xe @ w_ih.T + bias), written straight into a
        # double-buffered SBUF chunk (64 steps per chunk, no DRAM round-trip).
        TCH = 512 // BL  # 64 steps per chunk
        gi_buf = singles.tile([P, 2, TCH, NM, BL], BF16)
        p2ctx = ExitStack()
        p1psum = p2ctx.enter_context(tc.tile_pool(name="p1psum", bufs=2, space="PSUM"))

        def emit_p1_mms(c, m, r0, nr):
            # rows 512c+r0.. (+nr), gate-row tile m
            tag = f"p1ps{nr}"
            ps = p1psum.tile([P, nr], F32, tag=tag, name=f"p1ps_{c}_{m}_{r0}")
            for k in range(KI):
                nc.tensor.matmul(
                    ps[:],
                    lhsT=wih_sb[:, k, m, :],
                    rhs=xeT[:, c, k, r0:r0 + nr],
                    start=(k == 0), stop=(k == KI - 1),
                )
            return ps

        def emit_p1_bias(c, m, r0, nr, ps, on_act=False):
            dst = gi_buf[:, c % 2, r0 // BL:(r0 + nr) // BL, m, :]
            src = ps.rearrange("p (t b) -> p t b", b=BL)
            if on_act:
                nc.scalar.activation(dst, src, AF.Identity,
                                     bias=bgi_sb[:, m:m + 1])
            else:
                nc.vector.tensor_scalar_add(dst, src, bgi_sb[:, m:m + 1])

        # Chunk 0 is built in 4 waves of 16 steps: wave 0 before the loop,
        # waves 1-3 during early steps.  Chunks 1-3 stream in as half-units
        # during the preceding chunk.  mms go one step ahead of the bias so
        # the bias's psum wait is satisfied when the DVE sequencer hits it.
        p1_mm_sched, p1_bias_sched = {}, {}
        for c in range(1, S // TCH):
            for u in range(2 * NM):
                t0 = TCH * (c - 1) + (4 * u) // 3
                p1_mm_sched.setdefault(t0, []).append((c, u // 2, 256 * (u % 2), 256))
                p1_bias_sched.setdefault(t0 + 1, []).append((c, u // 2, 256 * (u % 2), 256))
        p1_pending = {}
        ps_q = []
        for u in range(2 * NM):
            m, h = u // 2, u % 2
            ps_q.append((m, h, emit_p1_mms(0, m, 256 * h, 256)))
            if len(ps_q) == 3:
                pm, ph, pps = ps_q.pop(0)
                emit_p1_bias(0, pm, 256 * ph, 256, pps, on_act=(pm % 2 == 1))
        for pm, ph, pps in ps_q:
            emit_p1_bias(0, pm, 256 * ph, 256, pps, on_act=(pm % 2 == 1))

        # ---- phase 2: recurrence ----
        gpool = ctx.enter_context(tc.tile_pool(name="gates", bufs=3))
        psg = p2ctx.enter_context(tc.tile_pool(name="psg", bufs=1, space="PSUM"))

        h0 = singles.tile([P, KH, BL], BF16)
        nc.vector.memset(h0[:], 0.0)
        h_prev = h0[:]  # [P, KH, BL]

        for t in range(S):
            gi_sb = gi_buf[:, (t // TCH) % 2, t % TCH].rearrange(
                "p m b -> p (m b)")  # [P, 192]

            # separate psum tiles per gate; burst ordered so ps_r lands
            # first (starts the ACT chain), then ps_n (t1), then ps_z.
            ps_r = psg.tile([P, 64], F32, tag="ps_r")
            ps_z = psg.tile([P, 64], F32, tag="ps_z")
            ps_n = psg.tile([P, 64], F32, tag="ps_n")

            # psum bank zeroing is lazy at whole-bank granularity on a
            # start=True, so the fold must come FIRST (start=True over the
            # whole tile); all gh matmuls then accumulate with start=False.
            def mm_group(pst, m0, fold_rhs):
                nc.tensor.matmul(pst[:], lhsT=iden_sb[:], rhs=fold_rhs,
                                 start=True, stop=False, skip_group_check=True)
                for m in range(m0, m0 + 8):
                    rel = BL * (m - m0)
                    for k in range(KH):
                        nc.tensor.matmul(
                            pst[:, rel:rel + BL],
                            lhsT=whh_sb[:, k, m, :],
                            rhs=h_prev[:, k, :],
                            start=False, stop=(m == m0 + 7 and k == KH - 1),
                            skip_group_check=True,
                        )

            mm_group(ps_r, 0, gi_sb[:, 0:64])
            mm_group(ps_n, 16, bhn_sb[:])
            mm_group(ps_z, 8, gi_sb[:, 64:128])

            # gates: ACT order r, z, tanh; critical chain
            # r -> t1 -> t2 -> tanh -> v -> h'  (t1 pure-SBUF via nb copy)
            r_g = gpool.tile([P, 64], BF16, tag="rg")
            nc.scalar.activation(r_g[:], ps_r[:], AF.Sigmoid)
            z_g = gpool.tile([P, 64], BF16, tag="zg")
            nc.scalar.activation(z_g[:], ps_z[:], AF.Sigmoid)
            t1 = gpool.tile([P, 64], BF16, tag="t1")
            nc.vector.tensor_mul(t1[:], r_g[:], ps_n[:])
            t2 = gpool.tile([P, 64], BF16, tag="t2")
            nc.vector.tensor_add(t2[:], t1[:], gi_sb[:, 128:192])
            n_g = gpool.tile([P, 64], BF16, tag="n")
            nc.scalar.activation(n_g[:], t2[:], AF.Tanh)
            omz = gpool.tile([P, 64], BF16, tag="omz")
            nc.vector.tensor_scalar(omz[:], z_g[:], -1.0, 1.0,
                                    op0=mybir.AluOpType.mult,
                                    op1=mybir.AluOpType.add)
            zh = gpool.tile([P, KH, BL], BF16, tag="zh")
            nc.vector.tensor_mul(
                zh[:], z_g[:].rearrange("p (k b) -> p k b", b=BL), h_prev)
            v_g = gpool.tile([P, 64], BF16, tag="v")
            nc.vector.tensor_mul(v_g[:], omz[:], n_g[:])
            h_new = outs_sb[:, :, t, :]
            nc.vector.tensor_add(
                h_new, v_g[:].rearrange("p (k b) -> p k b", b=BL), zh[:])

            # phase-1 work last: it then sits after this step's critical
            # ops in engine order
            for key in p1_bias_sched.get(t, ()):
                emit_p1_bias(*key, p1_pending.pop(key))
            for key in p1_mm_sched.get(t, ()):
                p1_pending[key] = emit_p1_mms(*key)

            h_prev = h_new

        p2ctx.close()

        # ---- phase 3: fc + log_softmax, batched per 512-row chunk ----
        # |logits| < ~4 so exp() is safe without the max-subtraction;
        # log_softmax = x - ln(sum(exp(x))).  Two passes keep Exp and Ln
        # table loads from thrashing.
        NQ = NT // 512
        with tc.tile_pool(name="p3", bufs=2) as p3, \
             tc.tile_pool(name="p3b", bufs=1) as p3b, \
             tc.tile_pool(name="p3psum", bufs=2, space="PSUM") as p3psum, \
             tc.tile_pool(name="p3psum2", bufs=1, space="PSUM") as p3psum2:
            lgps = [p3psum2.tile([P, 4, D_OUT], F32, tag=f"lgps{q}",
                                 name=f"lgps{q}") for q in range(NQ)]
            sms = [p3b.tile([P, 4], F32, tag=f"sm{q}", name=f"sm{q}")
                   for q in range(NQ)]
            for q in range(NQ):
                ps3 = p3psum.tile([D_OUT, 512], F32)
                for k in range(KH):
                    nc.tensor.matmul(
                        ps3[:],
                        lhsT=fcw_sb[:, k, :],
                        rhs=outs_sb[:, k, TCH * q:TCH * q + TCH, :].rearrange(
                            "p t b -> p (t b)"),
                        start=(k == 0), stop=(k == KH - 1),
                    )
                logitsT = p3.tile([D_OUT, 512], F32, tag="logitsT")
                nc.vector.tensor_scalar_add(logitsT[:], ps3[:], fcb_sb[:])
                for w in range(4):
                    nc.tensor.transpose(
                        lgps[q][:, w, :], logitsT[:, w * 128:w * 128 + 128],
                        iden32_sb[0:D_OUT, 0:D_OUT],
                    )
                ex = p3.tile([P, 4, D_OUT], BF16, tag="ex")
                nc.scalar.activation(ex[:], lgps[q][:], AF.Exp)
                nc.vector.reduce_sum(sms[q][:], ex[:], axis=mybir.AxisListType.X)
            for q in range(NQ):
                lsm = p3.tile([P, 4], F32, tag="lsm")
                nc.scalar.activation(lsm[:], sms[q][:], AF.Ln)
                lb = lsm[:]
                res = p3.tile([P, 4, D_OUT], F32, tag="res")
                nc.vector.tensor_sub(
                    res[:], lgps[q][:],
                    bass.AP(tensor=lb.tensor, offset=lb.offset,
                            ap=[list(lb.ap[0]), list(lb.ap[1]), [0, D_OUT]]),
                )
                nc.sync.dma_start(
                    bass.AP(tensor=out.tensor, offset=out.offset + q * 512 * D_OUT,
                            ap=[[D_OUT, P], [P * D_OUT, 4], [1, D_OUT]]),
                    res[:],
                )

    nc.compile()
    return nc


def assemble_output(core_outs):
    full = np.zeros((B, S, D_OUT), np.float32)
    for c, o in enumerate(core_outs):
        o = o.reshape(S, BL, D_OUT)  # rows are t-major: n = t*8 + b
        full[c * BL:(c + 1) * BL] = o.transpose(1, 0, 2)
    return full


# ----------------------------------------------------------------------------
# Harness entry point: kernel(**inputs) -> [B, S, D_OUT] float32
# ----------------------------------------------------------------------------
_CACHE = {}


def _get_nc():
    if "nc" not in _CACHE:
        _CACHE["nc"] = build_kernel()
    return _CACHE["nc"]


def kernel(x, emb, w_ih, w_hh, b_ih, b_hh, fc_w, fc_b):
    x = np.asarray(x)
    emb = np.asarray(emb, np.float32)
    w_ih = np.asarray(w_ih, np.float32)
    w_hh = np.asarray(w_hh, np.float32)
    b_ih = np.asarray(b_ih, np.float32)
    b_hh = np.asarray(b_hh, np.float32)
    fc_w = np.asarray(fc_w, np.float32)
    fc_b = np.asarray(fc_b, np.float32)

    from concourse.bass_utils import run_bass_kernel_spmd

    per_core = host_prep(x, emb, w_ih, w_hh, b_ih, b_hh, fc_w, fc_b)
    nc = _get_nc()
    res = run_bass_kernel_spmd(
        nc, per_core, core_ids=list(range(NCORES)), trace=False
    )
    return assemble_output([r["out"] for r in res.results])


# revision 53
# speedup vs baseline: 1.0086x; 1.0083x over previous
"""Bass/Tile GRU kernel for trn2, data-parallel over batch on 8 cores.

Model: xe = emb[x]; gi = xe @ w_ih.T + b_ih (+ b_hh for r,z); per step:
  gh = h @ w_hh.T
  r = sig(gi_r + gh_r); z = sig(gi_z + gh_z)
  n = tanh(gi_n + r * (gh_n + bhh_n)); h = (1-z)*n + z*h
then logits = outs @ fc_w.T + fc_b; out = log_softmax(logits)

Cost-model-driven layout (matmul cost = out free size; stationary side free):
all recurrence tensors live transposed, partition = hidden/gate-dim-in-tile,
free = (tile index k or m, batch b).  Per-core B_loc = 8.

  h      [128, 64]   h[j, b] at partition j%128, free (j//128)*8 + b
  psum   [128, 192]  ghT[g*1024+j, b] at partition j%128, free m*8+b,
                     m = (g*1024+j)//128  (m 0..7 r, 8..15 z, 16..23 n)
  gi_sb  [128, 192]  same layout as psum (from phase 1, via DRAM)

Recurrence matmuls are weights-stationary: lhsT = w_hh.T tile [128, 128]
(m,k), rhs = h slice [128, 8] -> 192 matmuls of out-free 8 per step, plus
two identity-lhsT fold matmuls that accumulate gi (r,z) and bhh_n into
psum.  h_new is produced directly in the h layout - no transposes.
"""

import numpy as np
from contextlib import ExitStack

import concourse.bass as bass
import concourse.tile as tile
from concourse import bacc, mybir

F32 = mybir.dt.float32
BF16 = mybir.dt.bfloat16
I16 = mybir.dt.int16
AF = mybir.ActivationFunctionType

VOCAB, D_IN, D_H, D_OUT, B, S = 32000, 512, 1024, 64, 64, 256
P = 128
NCORES = 8
BL = B // NCORES          # 8 batch rows per core
NT = BL * S               # 2048 (b,t) rows per core
FLUSH = 32                # steps between h flushes to DRAM
NM = 3 * D_H // P         # 24 gate-row tiles
KH = D_H // P             # 8 hidden k-tiles
KI = D_IN // P            # 4 input k-tiles
RC = 256                  # phase-1 row-chunk (32 steps * 8 batch)
NCH = NT // RC            # 8 phase-1 chunks


def host_prep(x, emb, w_ih, w_hh, b_ih, b_hh, fc_w, fc_b):
    """Produce the per-core and shared input arrays for the bass kernel."""
    import ml_dtypes
    emb_bf = emb.astype(ml_dtypes.bfloat16)
    # lhsT layouts: [p, k, m, mj]  ->  w[128*m+mj, 128*k+p]
    wih_l = np.ascontiguousarray(
        w_ih.T.reshape(KI, P, NM, P).transpose(1, 0, 2, 3)
    ).astype(ml_dtypes.bfloat16)
    whh_l = np.ascontiguousarray(
        w_hh.T.reshape(KH, P, NM, P).transpose(1, 0, 2, 3)
    ).astype(ml_dtypes.bfloat16)
    # bias for gi: b_ih everywhere, plus b_hh for the r,z gates (m < 16)
    bgiT = b_ih.reshape(NM, P).T.copy()
    bgiT[:, :16] += b_hh.reshape(NM, P).T[:, :16]
    bgiT = bgiT.astype(np.float32)  # [128, 24]
    # bhh_n expanded over batch: [p, 8k+b] = b_hh[2H + 128k + p]
    bhnE = np.repeat(
        b_hh[2 * D_H:].reshape(KH, P).T[:, :, None], BL, axis=2
    ).reshape(P, KH * BL).astype(ml_dtypes.bfloat16)  # [128, 64]
    fcw_l = np.ascontiguousarray(
        fc_w.T.reshape(KH, P, D_OUT)).astype(ml_dtypes.bfloat16)
    fcb = fc_b.astype(np.float32).reshape(D_OUT, 1)
    iden_bf = np.eye(P, dtype=np.float32).astype(ml_dtypes.bfloat16)
    iden32 = np.eye(P, dtype=np.float32)

    shared = dict(
        emb=emb_bf, wih=wih_l, whh=whh_l, bgi=bgiT, bhn=bhnE,
        fcw=fcw_l, fcb=fcb, iden=iden_bf, iden32=iden32,
    )
    per_core = []
    for c in range(NCORES):
        ids = np.ascontiguousarray(
            np.asarray(x[c * BL:(c + 1) * BL, :S]).T).reshape(-1).astype(np.int16)
        tmp = np.zeros((16, P), np.int16)
        i = np.arange(NT)
        li = i % 512
        tmp[li % 16, (i // 512) * 32 + li // 16] = ids
        idx = np.tile(tmp, (8, 1))  # replicated for the 8 Q7 cores
        per_core.append({"idx": idx, **shared})
    return per_core


def build_kernel():
    nc = bacc.Bacc("TRN2", debug=False, num_devices=1)

    idx = nc.dram_tensor("idx", [P, P], I16, kind="ExternalInput").ap()
    emb = nc.dram_tensor("emb", [VOCAB, D_IN], BF16, kind="ExternalInput").ap()
    wih = nc.dram_tensor("wih", [P, KI, NM, P], BF16, kind="ExternalInput").ap()
    whh = nc.dram_tensor("whh", [P, KH, NM, P], BF16, kind="ExternalInput").ap()
    bgi = nc.dram_tensor("bgi", [P, NM], F32, kind="ExternalInput").ap()
    bhn = nc.dram_tensor("bhn", [P, KH * BL], BF16, kind="ExternalInput").ap()
    fcw = nc.dram_tensor("fcw", [KH, P, D_OUT], BF16, kind="ExternalInput").ap()
    fcb = nc.dram_tensor("fcb", [D_OUT, 1], F32, kind="ExternalInput").ap()
    iden = nc.dram_tensor("iden", [P, P], BF16, kind="ExternalInput").ap()
    iden32 = nc.dram_tensor("iden32", [P, P], F32, kind="ExternalInput").ap()
    out = nc.dram_tensor("out", [NT, D_OUT], F32, kind="ExternalOutput").ap()

    with tile.TileContext(nc) as tc, ExitStack() as ctx:
        singles = ctx.enter_context(tc.tile_pool(name="singles", bufs=1))
        dram = ctx.enter_context(tc.tile_pool(name="dram", bufs=1, space="DRAM"))

        # ---- persistent SBUF state ----
        # order: what phase-1 chunk 0 needs comes first (idx, wih, bgi,
        # gathers); whh (6 MB, ~19 us) only gates step 0.
        idx_sb = singles.tile([P, P], I16)
        nc.sync.dma_start(idx_sb[:], idx)
        wih_sb = singles.tile([P, KI, NM, P], BF16)
        for mg in range(3):  # split so the first p1 units start sooner
            nc.sync.dma_start(wih_sb[:, :, 8 * mg:8 * mg + 8, :],
                              wih[:, :, 8 * mg:8 * mg + 8, :])
        bgi_sb = singles.tile([P, NM], F32)
        nc.sync.dma_start(bgi_sb[:], bgi)
        iden_sb = singles.tile([P, P], BF16)
        nc.sync.dma_start(iden_sb[:], iden)

        # gathered embeddings, transposed: xeT[p, gc, kc, i] = xe[512gc+i, 128kc+p]
        xeT = singles.tile([P, NT // 512, KI, 512], BF16)
        for gc in range(NT // 512):
            nc.gpsimd.dma_gather(
                out_ap=xeT[:, gc],
                in_ap=emb,
                idxs_ap=idx_sb[:, gc * 32:(gc + 1) * 32],
                num_idxs=512,
                num_idxs_reg=512,
                elem_size=D_IN,
                transpose=True,
            )

        # big weights go via the ACT hwdge queue, parallel to SP's wih
        whh_sb = singles.tile([P, KH, NM, P], BF16)
        nc.scalar.dma_start(whh_sb[:], whh)
        bhn_sb = singles.tile([P, KH * BL], BF16)
        nc.scalar.dma_start(bhn_sb[:], bhn)
        fcw_sb = singles.tile([P, KH, D_OUT], BF16)
        nc.scalar.dma_start(fcw_sb[:], fcw.rearrange("k p c -> p k c"))
        fcb_sb = singles.tile([D_OUT, 1], F32)
        nc.scalar.dma_start(fcb_sb[:], fcb)
        iden32_sb = singles.tile([P, P], F32)
        nc.scalar.dma_start(iden32_sb[:], iden32)

        # all hidden states live in SBUF: outs_sb[p, k, t, b] = h_t[128k+p, b]
        outs_sb = singles.tile([P, KH, S, BL], BF16)

        # ---- phase 1: giT = (xe @ w_ih.T + bias), written straight into a
        # double-buffered SBUF chunk (64 steps per chunk, no DRAM round-trip).
        TCH = 512 // BL  # 64 steps per chunk
        gi_buf = singles.tile([P, 2, TCH, NM, BL], BF16)
        p2ctx = ExitStack()
        p1psum = p2ctx.enter_context(tc.tile_pool(name="p1psum", bufs=3, space="PSUM"))

        def emit_p1_mms(c, m, r0, nr):
            # rows 512c+r0.. (+nr), gate-row tile m
            tag = f"p1ps{nr}"
            ps = p1psum.tile([P, nr], F32, tag=tag, name=f"p1ps_{c}_{m}_{r0}")
            for k in range(KI):
                nc.tensor.matmul(
                    ps[:],
                    lhsT=wih_sb[:, k, m, :],
                    rhs=xeT[:, c, k, r0:r0 + nr],
                    start=(k == 0), stop=(k == KI - 1),
                )
            return ps

        def emit_p1_bias(c, m, r0, nr, ps, on_act=False):
            dst = gi_buf[:, c % 2, r0 // BL:(r0 + nr) // BL, m, :]
            src = ps.rearrange("p (t b) -> p t b", b=BL)
            if on_act:
                nc.scalar.activation(dst, src, AF.Identity,
                                     bias=bgi_sb[:, m:m + 1])
            else:
                nc.vector.tensor_scalar_add(dst, src, bgi_sb[:, m:m + 1])

        # Chunk 0 is built in 4 waves of 16 steps: wave 0 before the loop,
        # waves 1-3 during early steps.  Chunks 1-3 stream in as half-units
        # during the preceding chunk.  mms go one step ahead of the bias so
        # the bias's psum wait is satisfied when the DVE sequencer hits it.
        p1_mm_sched, p1_bias_sched = {}, {}
        for c in range(1, S // TCH):
            for u in range(2 * NM):
                t0 = TCH * (c - 1) + (4 * u) // 3
                p1_mm_sched.setdefault(t0, []).append((c, u // 2, 256 * (u % 2), 256))
                p1_bias_sched.setdefault(t0 + 1, []).append((c, u // 2, 256 * (u % 2), 256))
        p1_pending = {}
        ps_q = []
        for u in range(2 * NM):
            m, h = u // 2, u % 2
            ps_q.append((m, h, emit_p1_mms(0, m, 256 * h, 256)))
            if len(ps_q) == 3:
                pm, ph, pps = ps_q.pop(0)
                emit_p1_bias(0, pm, 256 * ph, 256, pps, on_act=(pm % 2 == 1))
        for pm, ph, pps in ps_q:
            emit_p1_bias(0, pm, 256 * ph, 256, pps, on_act=(pm % 2 == 1))

        # ---- phase 2: recurrence ----
        gpool = ctx.enter_context(tc.tile_pool(name="gates", bufs=3))
        psg = p2ctx.enter_context(tc.tile_pool(name="psg", bufs=1, space="PSUM"))

        h0 = singles.tile([P, KH, BL], BF16)
        nc.vector.memset(h0[:], 0.0)
        h_prev = h0[:]  # [P, KH, BL]

        for t in range(S):
            gi_sb = gi_buf[:, (t // TCH) % 2, t % TCH].rearrange(
                "p m b -> p (m b)")  # [P, 192]

            # separate psum tiles per gate; burst ordered so ps_r lands
            # first (starts the ACT chain), then ps_n (t1), then ps_z.
            ps_r = psg.tile([P, 64], F32, tag="ps_r")
            ps_z = psg.tile([P, 64], F32, tag="ps_z")
            ps_n = psg.tile([P, 64], F32, tag="ps_n")

            # psum bank zeroing is lazy at whole-bank granularity on a
            # start=True, so the fold must come FIRST (start=True over the
            # whole tile); all gh matmuls then accumulate with start=False.
            def mm_group(pst, m0, fold_rhs):
                nc.tensor.matmul(pst[:], lhsT=iden_sb[:], rhs=fold_rhs,
                                 start=True, stop=False, skip_group_check=True)
                for m in range(m0, m0 + 8):
                    rel = BL * (m - m0)
                    for k in range(KH):
                        nc.tensor.matmul(
                            pst[:, rel:rel + BL],
                            lhsT=whh_sb[:, k, m, :],
                            rhs=h_prev[:, k, :],
                            start=False, stop=(m == m0 + 7 and k == KH - 1),
                            skip_group_check=True,
                        )

            mm_group(ps_r, 0, gi_sb[:, 0:64])
            mm_group(ps_n, 16, bhn_sb[:])
            mm_group(ps_z, 8, gi_sb[:, 64:128])

            # gates: ACT order r, z, tanh; critical chain
            # r -> t1 -> t2 -> tanh -> v -> h'  (t1 pure-SBUF via nb copy)
            r_g = gpool.tile([P, 64], BF16, tag="rg")
            nc.scalar.activation(r_g[:], ps_r[:], AF.Sigmoid)
            z_g = gpool.tile([P, 64], BF16, tag="zg")
            nc.scalar.activation(z_g[:], ps_z[:], AF.Sigmoid)
            t1 = gpool.tile([P, 64], BF16, tag="t1")
            nc.vector.tensor_mul(t1[:], r_g[:], ps_n[:])
            t2 = gpool.tile([P, 64], BF16, tag="t2")
            nc.vector.tensor_add(t2[:], t1[:], gi_sb[:, 128:192])
            n_g = gpool.tile([P, 64], BF16, tag="n")
            nc.scalar.activation(n_g[:], t2[:], AF.Tanh)
            omz = gpool.tile([P, 64], BF16, tag="omz")
            nc.vector.tensor_scalar(omz[:], z_g[:], -1.0, 1.0,
                                    op0=mybir.AluOpType.mult,
                                    op1=mybir.AluOpType.add)
            zh = gpool.tile([P, KH, BL], BF16, tag="zh")
            nc.vector.tensor_mul(
                zh[:], z_g[:].rearrange("p (k b) -> p k b", b=BL), h_prev)
            v_g = gpool.tile([P, 64], BF16, tag="v")
            nc.vector.tensor_mul(v_g[:], omz[:], n_g[:])
            h_new = outs_sb[:, :, t, :]
            nc.vector.tensor_add(
                h_new, v_g[:].rearrange("p (k b) -> p k b", b=BL), zh[:])

            # phase-1 work last: it then sits after this step's critical
            # ops in engine order
            for key in p1_bias_sched.get(t, ()):
                emit_p1_bias(*key, p1_pending.pop(key))
            for key in p1_mm_sched.get(t, ()):
                p1_pending[key] = emit_p1_mms(*key)

            h_prev = h_new

        p2ctx.close()

        # ---- phase 3: fc + log_softmax, batched per 512-row chunk ----
        # |logits| < ~4 so exp() is safe without the max-subtraction;
        # log_softmax = x - ln(sum(exp(x))).  Two passes keep Exp and Ln
        # table loads from thrashing.
        NQ = NT // 512
        with tc.tile_pool(name="p3", bufs=2) as p3, \
             tc.tile_pool(name="p3b", bufs=1) as p3b, \
             tc.tile_pool(name="p3psum", bufs=2, space="PSUM") as p3psum, \
             tc.tile_pool(name="p3psum2", bufs=1, space="PSUM") as p3psum2:
            lgps = [p3psum2.tile([P, 4, D_OUT], F32, tag=f"lgps{q}",
                                 name=f"lgps{q}") for q in range(NQ)]
            sms = [p3b.tile([P, 4], F32, tag=f"sm{q}", name=f"sm{q}")
                   for q in range(NQ)]
            for q in range(NQ):
                ps3 = p3psum.tile([D_OUT, 512], F32)
                for k in range(KH):
                    nc.tensor.matmul(
                        ps3[:],
                        lhsT=fcw_sb[:, k, :],
                        rhs=outs_sb[:, k, TCH * q:TCH * q + TCH, :].rearrange(
                            "p t b -> p (t b)"),
                        start=(k == 0), stop=(k == KH - 1),
                    )
                logitsT = p3.tile([D_OUT, 512], F32, tag="logitsT")
                nc.vector.tensor_scalar_add(logitsT[:], ps3[:], fcb_sb[:])
                for w in range(4):
                    nc.tensor.transpose(
                        lgps[q][:, w, :], logitsT[:, w * 128:w * 128 + 128],
                        iden32_sb[0:D_OUT, 0:D_OUT],
                    )
                ex = p3.tile([P, 4, D_OUT], BF16, tag="ex")
                nc.scalar.activation(ex[:], lgps[q][:], AF.Exp)
                nc.vector.reduce_sum(sms[q][:], ex[:], axis=mybir.AxisListType.X)
            for q in range(NQ):
                lsm = p3.tile([P, 4], F32, tag="lsm")
                nc.scalar.activation(lsm[:], sms[q][:], AF.Ln)
                lb = lsm[:]
                res = p3.tile([P, 4, D_OUT], F32, tag="res")
                nc.vector.tensor_sub(
                    res[:], lgps[q][:],
                    bass.AP(tensor=lb.tensor, offset=lb.offset,
                            ap=[list(lb.ap[0]), list(lb.ap[1]), [0, D_OUT]]),
                )
                nc.sync.dma_start(
                    bass.AP(tensor=out.tensor, offset=out.offset + q * 512 * D_OUT,
                            ap=[[D_OUT, P], [P * D_OUT, 4], [1, D_OUT]]),
                    res[:],
                )

    nc.compile()
    return nc


def assemble_output(core_outs):
    full = np.zeros((B, S, D_OUT), np.float32)
    for c, o in enumerate(core_outs):
        o = o.reshape(S, BL, D_OUT)  # rows are t-major: n = t*8 + b
        full[c * BL:(c + 1) * BL] = o.transpose(1, 0, 2)
    return full


# ----------------------------------------------------------------------------
# Harness entry point: kernel(**inputs) -> [B, S, D_OUT] float32
# ----------------------------------------------------------------------------
_CACHE = {}


def _get_nc():
    if "nc" not in _CACHE:
        _CACHE["nc"] = build_kernel()
    return _CACHE["nc"]


def kernel(x, emb, w_ih, w_hh, b_ih, b_hh, fc_w, fc_b):
    x = np.asarray(x)
    emb = np.asarray(emb, np.float32)
    w_ih = np.asarray(w_ih, np.float32)
    w_hh = np.asarray(w_hh, np.float32)
    b_ih = np.asarray(b_ih, np.float32)
    b_hh = np.asarray(b_hh, np.float32)
    fc_w = np.asarray(fc_w, np.float32)
    fc_b = np.asarray(fc_b, np.float32)

    from concourse.bass_utils import run_bass_kernel_spmd

    per_core = host_prep(x, emb, w_ih, w_hh, b_ih, b_hh, fc_w, fc_b)
    nc = _get_nc()
    res = run_bass_kernel_spmd(
        nc, per_core, core_ids=list(range(NCORES)), trace=False
    )
    return assemble_output([r["out"] for r in res.results])
